# revision 1
# baseline (speedup 1.0000x reference)
"""CrossHeadAttention Trainium2 kernel (8-core SPMD, data+head parallel).

Reference computation (per batch b):
    k = x_enc @ Wk ; v = x_enc @ Wv ; q = x @ Wq        (bias-free linears)
    wei = softmax((q @ k^T) / sqrt(1024))  per head
    out = wei @ v                                        -> [B, T, H, D]

Sharding: 8 cores = 2 batches x 4 head-groups (4 heads each). Each core
receives x[b], x_enc[b] and the 256-column slice of Wq/Wk/Wv for its heads,
and produces out[b][:, :, hg*4:(hg+1)*4, :]. No cross-core communication.

Per-core dataflow (matmuls in float32r = full-rate ~fp32):
  x_enc --PE transpose--> xeT[c,s] --W-stationary matmul--> kT[d,s], vT[d,s]
  x     --PE transpose--> xT[c,t]  -----------------------> qT[d,t]
  vT --PE transpose--> v[s,d] (+ones column for softmax sums)
  S^T[s,t] = k q^T   (K=64 contraction, 2 heads row-packed via tile_position)
  P^T = exp(S^T / 32) on ScalarE (scores are ~N(0,1): no max-subtraction)
  outT[d_aug,t] = v_aug.T @ P^T  (psum-accumulated over s; row 64 = sums)
  out[t,d] = PE-transpose(outT) * 1/sums  (DVE), DMA to HBM.

The transposed activations are built in 512-column chunks that feed their
projections immediately and die, so SBUF holds one rotating 16 KiB/partition
chunk pool instead of 64 KiB static buffers. The kernel runs as two phases
with scoped PSUM pools: a projection phase (6-bank rotating psum; psum->sbuf
rounding copies split between DVE and the otherwise-idle ScalarE) and an
attention phase (4 banks score double-buffer + 2 PV accumulators + 2
finalize banks), with the exp activation table preloaded at t=0.
"""

from contextlib import ExitStack

import numpy as np

import concourse.bacc as bacc
import concourse.tile as tile
from concourse import mybir
from concourse.bass_utils import run_bass_kernel_spmd
from concourse.masks import make_identity

# Problem constants (hardcoded per spec)
B = 2
T = 2048          # query length
S = 2048          # key/value length
C = 1024          # n_embd
H = 16            # total heads
D = 64            # head size
N_CORES = 8
HG = H // (N_CORES // B)       # heads per core = 4
DCORE = HG * D                 # 256 projected dims per core
P = 128                        # partitions
CT = C // P                    # 8 contraction tiles
NPAIR = HG // 2                # 2 head pairs per core
TCH = 512                      # t-chunk width in attention
NTCH = T // TCH                # 4
ST = S // P                    # 16 s-tiles

F32 = mybir.dt.float32
F32R = mybir.dt.float32r
AF = mybir.ActivationFunctionType

SCALE = float(C) ** -0.5       # 1/32, folded into the exp activation


def _build_chain(nc, rows, aux, xtp, src_dram, projs, identity, rowtag):
    """Stream src[t, c] through PE-transpose into rotating [c, 512] chunks,
    and run every projection in `projs` on each chunk as soon as it lands.

    projs: list of (w_slice [P, CT, P] f32r, out_slice_fn(chunk_idx) -> AP).
    """
    for sch in range(src_dram.shape[0] // 512):
        _build_chain_chunk(nc, rows, aux, xtp, src_dram, projs, identity,
                           rowtag, sch, act_copies=True)


def _chain_chunk_pieces(nc, rows, aux, xtp, src_dram, projs, identity,
                        rowtag, sch, act_copies=False):
    """Emission pieces for one 512-wide x^T chunk + its projections.

    Returns a list of zero-arg callables; calling them in order (possibly
    interleaved with other emission) builds the chunk. When act_copies is
    set, half the psum->sbuf copies go to ScalarE instead of DVE (used
    pre-attention while ScalarE is otherwise idle).
    """
    state = {}

    def row_piece(r4):
        def go():
            if r4 == 0:
                state["xc"] = xtp.tile([P, CT, 512], F32R, tag="xch",
                                       name="xch")
            r = sch * 4 + r4
            row = rows.tile([P, C], F32, tag=rowtag, name="row")
            nc.sync.dma_start(out=row, in_=src_dram[r * P:(r + 1) * P, :])
            for cq in range(CT // 4):
                tp = aux.tile([P, 4 * P], F32, tag="aux", name="tp")
                for j in range(4):
                    ct = 4 * cq + j
                    nc.tensor.transpose(
                        tp[:, j * P:(j + 1) * P],
                        row[:, ct * P:(ct + 1) * P], identity)
                if act_copies and cq % 2:
                    copy_fn = nc.scalar.copy
                else:
                    copy_fn = lambda out, in_: nc.vector.tensor_copy(
                        out=out, in_=in_)
                copy_fn(
                    out=state["xc"][:, 4 * cq:4 * cq + 4,
                                    r4 * P:(r4 + 1) * P],
                    in_=tp.rearrange("p (j t) -> p j t", j=4))
        return go

    def proj_piece(w_slice, out_fn):
        def go():
            ps = aux.tile([P, 512], F32, tag="aux", name="ps")
            for ct in range(CT):
                nc.tensor.matmul(
                    ps, w_slice[:, ct, :], state["xc"][:, ct, :],
                    start=(ct == 0), stop=(ct == CT - 1))
            nc.vector.tensor_copy(out=out_fn(sch), in_=ps)
        return go

    return [row_piece(r4) for r4 in range(4)] +            [proj_piece(w, f) for w, f in projs]


def _build_chain_chunk(nc, rows, aux, xtp, src_dram, projs, identity,
                       rowtag, sch, act_copies=False):
    for piece in _chain_chunk_pieces(nc, rows, aux, xtp, src_dram, projs,
                                     identity, rowtag, sch, act_copies):
        piece()


def _build_v_transpose(nc, aux, vT, v_sb, identity, pt):
    """v_sb[s, 2pt:2pt+2, d] = (vT pair tile)^T via PE transpose."""
    for sq in range(ST // 4):
        tp = aux.tile([P, 4 * P], F32, tag="aux", name="tpv")
        for j in range(4):
            st = 4 * sq + j
            nc.tensor.transpose(
                tp[:, j * P:(j + 1) * P],
                vT.bitcast(F32)[:, st * P:(st + 1) * P], identity)
        for j in range(4):
            st = 4 * sq + j
            nc.vector.tensor_copy(
                out=v_sb[:, st, 2 * pt:2 * pt + 2, 0:D],
                in_=tp[:, j * P:(j + 1) * P].rearrange(
                    "p (h d) -> p h d", h=2))


def _build_attention_tch(nc, spsum, pvpools, aux, psb, otp, fin,
                         kT, qT, v_sb, identity, out, pair, tch,
                         interleave=()):
    """Attention st-loop for one head pair and one t-chunk -> oT tiles.

    `interleave`: emission pieces (e.g. next chunk's build) spliced between
    st iterations so the static schedule overlaps them with the exp stream.
    """
    if True:
        interleave = list(interleave)
        tsl = slice(tch * TCH, (tch + 1) * TCH)
        pv_ps = [pvpools[h2].tile([D + 1, TCH], F32, tag=f"pv{h2}",
                                  name=f"pv{h2}")
                 for h2 in range(2)]
        for st in range(ST):
            s_ps = spsum.tile([P, 2 * TCH], F32, tag="s", name="s_ps")
            for h2 in range(2):
                nc.tensor.matmul(
                    s_ps[:, h2 * TCH:(h2 + 1) * TCH],
                    kT[h2 * D:(h2 + 1) * D, pair, st * P:(st + 1) * P],
                    qT[h2 * D:(h2 + 1) * D, pair, tsl],
                    start=True, stop=True,
                    tile_position=(h2 * D, 0),
                )
            p_sb = psb.tile([P, 2 * TCH], F32R, tag="p", name="p_sb")
            nc.scalar.activation(out=p_sb, in_=s_ps, func=AF.Exp, scale=SCALE)
            for h2 in range(2):
                nc.tensor.matmul(
                    pv_ps[h2],
                    v_sb[:, st, 2 * pair + h2, :],
                    p_sb[:, h2 * TCH:(h2 + 1) * TCH],
                    start=(st == 0), stop=(st == ST - 1),
                )
            if interleave and st % 2 == 1:
                interleave.pop(0)()
        for piece in interleave:
            piece()
        oT = []
        for h2 in range(2):
            t_ = otp.tile([D + 1, TCH], F32, tag=f"oT{pair}{h2}",
                          name=f"oT{pair}{h2}")
            nc.vector.tensor_copy(out=t_, in_=pv_ps[h2])
            oT.append(t_)
        return oT


def _build_finalize_tch(nc, spsum, fin, oT, identity, out, pair, tch):
    """Transpose oT heads into a spsum bank, normalize by sums, store.

    Uses the spsum pool (not aux) so the next chunk-build's transposes are
    never serialized behind this tail work.
    """
    for sub in range(TCH // P):
        tt = tch * (TCH // P) + sub
        o_tile = fin.tile([P, 2 * D], F32, tag="o", name="o_tile")
        tp = spsum.tile([P, 2 * (D + 1)], F32, tag="ft", name="ft")
        for h2 in range(2):
            nc.tensor.transpose(
                tp[:, h2 * (D + 1):(h2 + 1) * (D + 1)],
                oT[h2][:, sub * P:(sub + 1) * P],
                identity[0:D + 1, 0:D + 1])
        tph = tp.rearrange("p (h e) -> p h e", h=2)
        r2 = fin.tile([P, 2], F32, tag="r", name="r2")
        nc.vector.reciprocal(out=r2, in_=tph[:, :, D])
        for h2 in range(2):
            nc.vector.tensor_scalar_mul(
                out=o_tile[:, h2 * D:(h2 + 1) * D],
                in0=tph[:, h2, 0:D], scalar1=r2[:, h2:h2 + 1])
        # SWDGE: keeps this dependent store out of SP's in-order
        # stream so it cannot head-of-line-block later row loads
        nc.gpsimd.dma_start(
            out=out[tt * P:(tt + 1) * P,
                    pair * 2 * D:(pair + 1) * 2 * D],
            in_=o_tile)


def _attention_phase(nc, tc, kT, qT, v_sb, identity, out,
                     psb, otp, fin):
    with tc.tile_pool(name="spsum", bufs=2, space="PSUM") as spsum, \
         tc.tile_pool(name="pvpsum0", bufs=1, space="PSUM") as pvp0, \
         tc.tile_pool(name="pvpsum1", bufs=1, space="PSUM") as pvp1, \
         tc.tile_pool(name="ftpsum", bufs=2, space="PSUM") as ftp:
        pvpools = (pvp0, pvp1)
        for tch in range(NTCH):
            oT0 = _build_attention_tch(
                nc, spsum, pvpools, None, psb, otp, fin,
                kT, qT, v_sb, identity, out, 0, tch)
            # pair-0 finalize emitted before pair-1 attention so its
            # transposes/stores run under pair-1's exp stream
            _build_finalize_tch(nc, ftp, fin, oT0, identity, out, 0, tch)
            oT1 = _build_attention_tch(
                nc, spsum, pvpools, None, psb, otp, fin,
                kT, qT, v_sb, identity, out, 1, tch)
            _build_finalize_tch(nc, ftp, fin, oT1, identity, out, 1, tch)


def _build_body(nc, tc, x, xe, wq, wk, wv, out):
    with ExitStack() as ctx:
        consts = ctx.enter_context(tc.tile_pool(name="consts", bufs=1))
        big = ctx.enter_context(tc.tile_pool(name="big", bufs=1))
        psb = ctx.enter_context(tc.tile_pool(name="psb", bufs=3))
        otp = ctx.enter_context(tc.tile_pool(name="otp", bufs=2))
        fin = ctx.enter_context(tc.tile_pool(name="fin", bufs=3))

        identity = consts.tile([P, P], F32)
        make_identity(nc, identity)
        # prime the ScalarE exp table at t=0 so the ~2.7us ACT_TABLE_LOAD is
        # off the critical path of the first real exp
        dummy = consts.tile([1, 2], F32)
        nc.vector.memset(dummy, 0.0)
        nc.scalar.activation(out=dummy, in_=dummy, func=AF.Exp)

        kT = big.tile([P, NPAIR, S], F32R, tag="kT")
        qT = big.tile([P, NPAIR, T], F32R, tag="qT")
        vT0 = big.tile([P, S], F32R, tag="vT0")
        vT1 = big.tile([P, S], F32R, tag="vT1")
        # v, with a ones column appended per head (col D) for softmax sums
        v_sb = big.tile([P, ST, HG, D + 1], F32R, tag="v_sb")
        nc.vector.memset(v_sb[:, :, :, D].bitcast(F32), 1.0)

        with tc.tile_pool(name="xtp", bufs=2) as xtp, \
             tc.tile_pool(name="rows", bufs=3) as rows, \
             tc.tile_pool(name="wpool", bufs=1) as wpool:

            # weights: DMA f32 staging -> DVE rounding copy -> f32r
            w_sbs = {}
            for name, wdram in (("wk", wk), ("wv", wv), ("wq", wq)):
                stage = wpool.tile([P, CT, DCORE], F32, tag="wstage",
                                   name="wstage")
                nc.gpsimd.dma_start(
                    out=stage, in_=wdram.rearrange("(ct p) d -> p ct d", p=P))
                wsb = wpool.tile([P, CT, DCORE], F32R, tag=f"{name}_sb",
                                 name=f"{name}_sb")
                nc.vector.tensor_copy(out=wsb, in_=stage)
                w_sbs[name] = wsb

            def _dsl(wname, dt_):
                return w_sbs[wname][:, :, dt_ * P:(dt_ + 1) * P]

            with tc.tile_pool(name="chainps", bufs=6, space="PSUM") as aux:
                # xe chain: k^T and v^T for both pairs, chunk-streamed
                _build_chain(
                    nc, rows, aux, xtp, xe,
                    [(_dsl("wk", 0),
                      lambda s: kT[:, 0, s * 512:(s + 1) * 512]),
                     (_dsl("wv", 0),
                      lambda s: vT0[:, s * 512:(s + 1) * 512]),
                     (_dsl("wk", 1),
                      lambda s: kT[:, 1, s * 512:(s + 1) * 512]),
                     (_dsl("wv", 1),
                      lambda s: vT1[:, s * 512:(s + 1) * 512])],
                    identity, "row")
                _build_v_transpose(nc, aux, vT0, v_sb, identity, 0)
                _build_v_transpose(nc, aux, vT1, v_sb, identity, 1)

                # x chain: q^T for both pairs
                qproj = [(_dsl("wq", 0),
                          lambda s: qT[:, 0, s * 512:(s + 1) * 512]),
                         (_dsl("wq", 1),
                          lambda s: qT[:, 1, s * 512:(s + 1) * 512])]
                _build_chain(nc, rows, aux, xtp, x, qproj, identity, "row")

            _attention_phase(nc, tc, kT, qT, v_sb, identity, out,
                             psb, otp, fin)


def build_program():
    nc = bacc.Bacc("TRN2", target_bir_lowering=False, debug=False,
                   num_devices=N_CORES)

    x = nc.dram_tensor("x", [T, C], F32, kind="ExternalInput").ap()
    xe = nc.dram_tensor("xe", [S, C], F32, kind="ExternalInput").ap()
    wq = nc.dram_tensor("wq", [C, DCORE], F32, kind="ExternalInput").ap()
    wk = nc.dram_tensor("wk", [C, DCORE], F32, kind="ExternalInput").ap()
    wv = nc.dram_tensor("wv", [C, DCORE], F32, kind="ExternalInput").ap()
    out = nc.dram_tensor("out", [T, DCORE], F32, kind="ExternalOutput").ap()

    with tile.TileContext(nc) as tc:
        _build_body(nc, tc, x, xe, wq, wk, wv, out)
    nc.compile()
    return nc


_NC_CACHE = None


def _get_program():
    global _NC_CACHE
    if _NC_CACHE is None:
        _NC_CACHE = build_program()
    return _NC_CACHE


def kernel(x_enc, x, Wk, Wq, Wv):
    x_enc = np.asarray(x_enc, dtype=np.float32)
    x = np.asarray(x, dtype=np.float32)
    Wk = np.asarray(Wk, dtype=np.float32)
    Wq = np.asarray(Wq, dtype=np.float32)
    Wv = np.asarray(Wv, dtype=np.float32)

    nc = _get_program()
    in_maps = []
    for core in range(N_CORES):
        b, hg = divmod(core, N_CORES // B)
        csl = slice(hg * DCORE, (hg + 1) * DCORE)
        in_maps.append({
            "x": np.ascontiguousarray(x[b]),
            "xe": np.ascontiguousarray(x_enc[b]),
            "wq": np.ascontiguousarray(Wq[:, csl]),
            "wk": np.ascontiguousarray(Wk[:, csl]),
            "wv": np.ascontiguousarray(Wv[:, csl]),
        })
    res = run_bass_kernel_spmd(nc, in_maps, list(range(N_CORES)))

    full = np.empty((B, T, H, D), dtype=np.float32)
    for core in range(N_CORES):
        b, hg = divmod(core, N_CORES // B)
        o = res.results[core]["out"].reshape(T, HG, D)
        full[b, :, hg * HG:(hg + 1) * HG, :] = o
    return full



# revision 8
# speedup vs baseline: 1.0987x; 1.0987x over previous
"""CrossHeadAttention Trainium2 kernel v2 (8-core SPMD, data+head parallel).

Reference computation (per batch b):
    k = x_enc @ Wk ; v = x_enc @ Wv ; q = x @ Wq        (bias-free linears)
    wei = softmax((q @ k^T) / sqrt(1024))  per head
    out = wei @ v                                        -> [B, T, H, D]

Sharding: 8 cores = 2 batches x 4 head-groups (4 heads each). No cross-core
communication.

Design vs v1 (260.7us): the ScalarE exp stream (128 instrs x [128,1024],
~134us busy) is the hard floor of this algorithm, so everything else is
restructured to hide under it:
  - all matmuls in bf16 (measured end-to-end err 2.9e-3 < 2e-2 gate; fp8
    measured 2-3e-2 -> rejected)
  - x/xe/v transposes ride the DMA xbar (dma_start_transpose, 14ns per
    16x128 tile) with f32->bf16 rounding on the otherwise-idle Pool engine;
    the PE does no transposes at all
  - PV uses p-STATIONARY matmuls: out[t,d+1] accumulates over s-tiles with
    65-col moving v_aug instead of 512-col moving p: PV 131K -> 67K cycles,
    and the output lands [t, d] so no finalize transposes
  - every projection is split into per-rowblock 128-col strips (8 matmuls
    each) so pieces are small enough to schedule smoothly and the first
    kT/qT columns appear with minimum latency
  - all 32 row DMA+round+transpose chains are issued up front (every chunk
    has its own SBUF buffer) so spliced projection matmuls never block the
    in-order PE stream on DMA latency
  - PV(st) trails exp(st) by PV_LAG slots to shift PE work out of the
    xe-heavy strip 0
PSUM: lead-in uses a scoped pool (closed before strips); strips use 2x
score double-buffer [128,1024] (4 banks) + 3 PV banks (16 accumulation
groups of [128,65] packed 6/6/4) + 1 chain bank for spliced projections.
"""

from contextlib import ExitStack

import numpy as np

import concourse.bacc as bacc
import concourse.tile as tile
from concourse import mybir
from concourse.bass_utils import run_bass_kernel_spmd
from concourse.masks import make_identity

# Problem constants (hardcoded per spec)
B = 2
T = 2048          # query length
S = 2048          # key/value length
C = 1024          # n_embd
H = 16            # total heads
D = 64            # head size
N_CORES = 8
HG = H // (N_CORES // B)       # heads per core = 4
DCORE = HG * D                 # 256 projected dims per core
P = 128                        # partitions
CT = C // P                    # 8 contraction tiles
NPAIR = HG // 2                # 2 head pairs per core
TCH = 512                      # t-chunk width (strip)
NTCH = T // TCH                # 4 strips
ST = S // P                    # 16 s-tiles
NTB = TCH // P                 # 4 t-blocks of 128 per strip

F32 = mybir.dt.float32
BF16 = mybir.dt.bfloat16
AF = mybir.ActivationFunctionType

SCALE = float(C) ** -0.5       # 1/32, folded into the exp activation

# PV psum packing: 16 groups g = head*4 + tb over 3 banks (6/6/4)
PV_BANK = (0, 0, 0, 0, 0, 0, 1, 1, 1, 1, 1, 1, 2, 2, 2, 2)
PV_SLOT = (0, 1, 2, 3, 4, 5, 0, 1, 2, 3, 4, 5, 0, 1, 2, 3)
PV_CNT = (6, 6, 4)             # groups per bank
PV_LAG = 8                     # PV trails exp by this many st-slots


def _build_body(nc, tc, x, xe, wq, wk, wv, out):
    with ExitStack() as ctx:
        consts = ctx.enter_context(tc.tile_pool(name="consts", bufs=1))
        wpool = ctx.enter_context(tc.tile_pool(name="wpool", bufs=1))
        rows = ctx.enter_context(tc.tile_pool(name="rows", bufs=10))
        rbf = ctx.enter_context(tc.tile_pool(name="rbf", bufs=8))
        xtp = ctx.enter_context(tc.tile_pool(name="xtp", bufs=1))
        big = ctx.enter_context(tc.tile_pool(name="big", bufs=1))
        wstage = ctx.enter_context(tc.tile_pool(name="wstage", bufs=1))

        # prime the ScalarE exp table at t=0 so the ~2.7us ACT_TABLE_LOAD is
        # off the critical path of the first real exp
        dummy = consts.tile([1, 2], F32, name="dummy")
        nc.vector.memset(dummy, 0.0)
        nc.scalar.activation(out=dummy, in_=dummy, func=AF.Exp)

        # big persistent tensors (bf16)
        kT = big.tile([P, NPAIR, S], BF16, tag="kT", name="kT")
        qT = big.tile([P, NPAIR, T], BF16, tag="qT", name="qT")
        vT = [big.tile([P, S], BF16, tag=f"vT{pr}", name=f"vT{pr}")
              for pr in range(NPAIR)]
        # v in [s, st, head, d] layout with a ones column at d=64 for the
        # softmax denominators (PV moving tensor is [128, 65])
        v_sb = big.tile([P, ST, HG, D + 1], BF16, tag="v_sb", name="v_sb")
        nc.vector.memset(v_sb[:, :, :, D], 1.0)

        # weights: DMA f32 staging -> rounding copy -> bf16
        w_sbs = {}

        def load_w_dma(name, wdram):
            stage = wstage.tile([P, CT, DCORE], F32, tag=f"wstage_{name}",
                                name=f"wstage_{name}")
            nc.sync.dma_start(
                out=stage, in_=wdram.rearrange("(ct p) d -> p ct d", p=P))
            w_sbs[f"{name}_stage"] = stage

        def load_w_round(name, eng):
            wsb = wpool.tile([P, CT, DCORE], BF16, tag=f"{name}_sb",
                             name=f"{name}_sb")
            eng.tensor_copy(out=wsb, in_=w_sbs[f"{name}_stage"])
            w_sbs[name] = wsb

        def wsl(wname, pair):
            return w_sbs[wname][:, :, pair * P:(pair + 1) * P]

        # ---- chunk machinery -------------------------------------------
        # A "chunk" is 512 rows of x or xe: 4 rowblocks, each DMA f32 ->
        # Pool bf16 round -> DMA xbar transpose into the chunk's private
        # xcT buffer. Projections run as per-rowblock 128-col strips (8
        # accumulating matmuls sharing a psum bank, one column group per
        # rowblock) + a DVE bf16 copy per strip.
        xc_state = {}
        cur_ps = [None]

        def row_dma_piece(src_dram, sch, r4, ctag):
            def go():
                row = rows.tile([P, C], F32, tag="row", name="row")
                xc_state[(ctag, "row", r4)] = row
                nc.sync.dma_start(out=row, in_=src_dram[
                    (sch * 4 + r4) * P:(sch * 4 + r4 + 1) * P, :])
            return go

        def row_piece(sch, r4, ctag):
            # the xbar transpose requires a CONTIGUOUS output tile on real
            # hardware (strided 3D out APs silently corrupt), so each
            # rowblock transposes into its own [P, CT, P] tile
            def go():
                xcb = xtp.tile([P, CT, P], BF16, tag="xch", bufs=16,
                               name=f"xch_{ctag}_{r4}")
                xc_state[(ctag, "xc", r4)] = xcb
                row = xc_state.pop((ctag, "row", r4))
                rb = rbf.tile([P, C], BF16, tag="rbf", name="rbf")
                # alternate rounding between Pool and DVE so neither paces
                # the transpose pipeline
                eng = nc.gpsimd if r4 % 2 else nc.vector
                eng.tensor_copy(out=rb, in_=row)
                nc.sync.dma_start_transpose(xcb, rb)
            return go

        # strip psum slices: list of [P, P] f32 bank slices + rotation idx.
        # During the lead and early strip 0 (before PV needs its banks) the
        # idle pv2 bank is borrowed, doubling the rotation depth.
        strip_rot = [[], 0]
        in_lead = [True]

        def proj_strip(wname, pair, dest_fn, ctag, r4):
            """One 128-col strip of a projection: 8 accumulating matmuls
            into a rotating 128-col slice of a persistent psum bank tile
            (PSUM pool slots are bank-granular, so sub-bank buffers must be
            hand-sliced), then a DVE bf16 copy. The 4-deep slice rotation
            keeps the PE from stalling on the previous strip's copy."""
            def go():
                slices = strip_rot[0]
                sl = strip_rot[1] % len(slices)
                ps = slices[sl]
                strip_rot[1] = (sl + 1) % len(slices)
                w = wsl(wname, pair)
                xc = xc_state[(ctag, "xc", r4)]
                for ct in range(CT):
                    nc.tensor.matmul(ps, w[:, ct, :], xc[:, ct, :],
                                     start=(ct == 0), stop=(ct == CT - 1),
                                     skip_group_check=True)
                nc.vector.tensor_copy(out=dest_fn(r4), in_=ps)
            return go

        def k_dest(pr, sch):
            return lambda r4: kT[:, pr, sch * TCH + r4 * P:
                                 sch * TCH + (r4 + 1) * P]

        def q_dest(pr, sch):
            return lambda r4: qT[:, pr, sch * TCH + r4 * P:
                                 sch * TCH + (r4 + 1) * P]

        def v_dest(pr, sch):
            return lambda r4: vT[pr][:, sch * TCH + r4 * P:
                                     sch * TCH + (r4 + 1) * P]

        def v_transpose_piece(pair, sch):
            # contiguous xbar target, then a (2x-mode) DVE copy into the
            # strided [s, st, head, 65] PV layout
            def go():
                v4 = rbf.tile([P, 4, P], BF16, tag="v4", name="v4")
                nc.sync.dma_start_transpose(
                    v4, vT[pair][:, sch * TCH:(sch + 1) * TCH])
                nc.vector.tensor_copy(
                    out=v_sb[:, sch * 4:(sch + 1) * 4,
                             2 * pair:2 * pair + 2, 0:D],
                    in_=v4.rearrange("p q (h d) -> p q h d", h=2))
            return go

        def row_dmas(src, sch, ctag):
            return [row_dma_piece(src, sch, r4, ctag) for r4 in range(4)]

        def row_chains(sch, ctag):
            return [row_piece(sch, r4, ctag) for r4 in range(4)]

        def proj_strips(wname, dest_mk, ctag, sch):
            return [proj_strip(wname, pr, dest_mk(pr, sch), ctag, r4)
                    for pr in range(NPAIR) for r4 in range(4)]

        spsum = ctx.enter_context(
            tc.tile_pool(name="spsum", bufs=2, space="PSUM"))
        pvpsum = ctx.enter_context(
            tc.tile_pool(name="pvpsum", bufs=1, space="PSUM"))
        chain = ctx.enter_context(
            tc.tile_pool(name="chainps", bufs=1, space="PSUM"))
        chain_tile = chain.tile([P, 4 * P], F32, tag="chain",
                                name="chain_strip")
        prepv = pvpsum.tile([P, TCH], F32, tag="pv2", name="prepv")
        strip_rot[0] = [chain_tile[:, i * P:(i + 1) * P] for i in range(4)] \
            + [prepv[:, i * P:(i + 1) * P] for i in range(4)]
        strip_rot[1] = 0
        in_lead[0] = False

        # ---- prefetch + lead-in ----------------------------------------
        # Everything rides the DMA xbar; the wire order IS the schedule.
        # exp(0,0) needs: x0 chain + Wq (qT strip 0) and xe0 rowblock 0 +
        # Wk (kT s-tile 0). Chunk c+1 row DMAs are issued before chunk c's
        # round/transpose pieces so the wire never waits on engine latency.
        x0_d = row_dmas(x, 0, "x0")
        xe0_d = row_dmas(xe, 0, "xe0")
        x0_c = row_chains(0, "x0")
        xe0_c = row_chains(0, "xe0")
        for p_ in x0_d:
            p_()
        xe0_d[0]()
        load_w_dma("wq", wq)
        load_w_dma("wk", wk)
        for p_ in x0_c:
            p_()
        xe0_c[0]()
        for p_ in xe0_d[1:]:
            p_()
        load_w_round("wq", nc.vector)
        load_w_round("wk", nc.vector)
        for p_ in xe0_c[1:]:
            p_()
        for p_ in row_dmas(xe, 1, "xe1"):
            p_()
        load_w_dma("wv", wv)
        for p_ in proj_strips("wq", q_dest, "x0", 0):
            p_()
        k0 = proj_strips("wk", k_dest, "xe0", 0)
        k0[0]()   # pair0 r0
        k0[4]()   # pair1 r0
        load_w_round("wv", nc.vector)
        for p_ in row_chains(1, "xe1"):
            p_()
        for p_ in row_dmas(xe, 2, "xe2"):
            p_()
        for p_ in row_chains(2, "xe2"):
            p_()
        for p_ in row_dmas(xe, 3, "xe3"):
            p_()
        for p_ in row_chains(3, "xe3"):
            p_()
        ppool = ctx.enter_context(tc.tile_pool(name="ppool", bufs=16))
        rpool = ctx.enter_context(tc.tile_pool(name="rpool", bufs=4))
        opool = ctx.enter_context(tc.tile_pool(name="opool", bufs=3))


        # splice schedule: slot (tch*16+st) -> remaining projection pieces,
        # placed JUST-IN-TIME at their deadline (earlier placement risks
        # head-of-line blocking the in-order PE stream on DMA arrival):
        #   kT chunk c -> slot 4c (before that slot's scores)
        #   vT chunk c -> slot 4c+PV_LAG (before that slot's PV flush)
        #   qT chunk c -> slot 16c-1
        def vpieces(c):
            ps = proj_strips("wv", v_dest, f"xe{c}", c)
            return ps[0:4] + [v_transpose_piece(0, c)] + ps[4:8] + \
                [v_transpose_piece(1, c)]

        sched = {}

        def put(slot, pieces):
            sched.setdefault(slot, []).extend(pieces)

        put(0, [k0[1], k0[5]])
        put(1, [k0[2], k0[6]])
        put(2, [k0[3], k0[7]])
        put(3, proj_strips("wk", k_dest, "xe1", 1))
        put(7, proj_strips("wk", k_dest, "xe2", 2))
        put(11, proj_strips("wk", k_dest, "xe3", 3))
        import os
        VK = int(os.environ.get("VK", "6"))
        v0p = vpieces(0)
        put(max(VK - 1, 0), v0p[0:5])
        put(VK, v0p[5:10])
        for c in range(1, 4):
            vp = vpieces(c)
            put(4 * c + VK - 1, vp[0:5])
            put(4 * c + VK, vp[5:10])
        for c in range(1, 4):
            base = 16 * c
            put(base - 10, row_dmas(x, c, f"x{c}"))
            put(base - 8, row_chains(c, f"x{c}"))
            put(base - 3, proj_strips("wq", q_dest, f"x{c}", c))

        # ---- attention strips ------------------------------------------
        def scores(tch, st, pair, s_ps):
            for h2 in range(2):
                nc.tensor.matmul(
                    s_ps[:, h2 * TCH:(h2 + 1) * TCH],
                    kT[h2 * D:(h2 + 1) * D, pair, st * P:(st + 1) * P],
                    qT[h2 * D:(h2 + 1) * D, pair,
                       tch * TCH:(tch + 1) * TCH],
                    start=True, stop=True,
                    tile_position=(h2 * D, 0))

        def pv8(st, pair, p_t, pv_banks):
            for h2 in range(2):
                h = 2 * pair + h2
                for tb in range(NTB):
                    g = h * NTB + tb
                    bank = pv_banks[PV_BANK[g]]
                    sl = PV_SLOT[g]
                    # HW: one open accumulation group per bank at a time,
                    # so packed groups must never use start=True; the banks
                    # are DVE-memset once per strip instead and every matmul
                    # accumulates (verified exact on hardware).
                    nc.tensor.matmul(
                        bank[:, sl * (D + 1):(sl + 1) * (D + 1)],
                        p_t[:, h2 * TCH + tb * P:h2 * TCH + (tb + 1) * P],
                        v_sb[:, st, h, :],
                        start=False, stop=(st == ST - 1),
                        skip_group_check=True)

        def normalize_store(tch, pv_banks):
            recs = rpool.tile([P, 3, 6], F32, tag="rec", name="rec")
            for b_ in range(3):
                cnt = PV_CNT[b_]
                nc.vector.reciprocal(
                    out=recs[:, b_, 0:cnt],
                    in_=pv_banks[b_][:, 0:cnt * (D + 1)].rearrange(
                        "p (g e) -> p g e", e=D + 1)[:, :, D])
            for tb in range(NTB):
                o_sb = opool.tile([P, HG, D], F32, tag="osb", name="osb")
                for h in range(HG):
                    g = h * NTB + tb
                    bank = pv_banks[PV_BANK[g]]
                    sl = PV_SLOT[g]
                    nc.vector.tensor_scalar_mul(
                        out=o_sb[:, h, :],
                        in0=bank[:, sl * (D + 1):sl * (D + 1) + D],
                        scalar1=recs[:, PV_BANK[g], sl:sl + 1])
                tt = tch * NTB + tb
                nc.sync.dma_start(out=out[tt * P:(tt + 1) * P, :], in_=o_sb)

        def alloc_pv_banks():
            banks = [pvpsum.tile([P, TCH], F32, tag=f"pv{b_}",
                                 name=f"pv{b_}") for b_ in range(3)]
            for b_ in banks:
                nc.vector.memset(b_, 0.0)
            return banks

        pending = []     # [(slot, st, pair, p_tile)] awaiting PV
        pv_banks = None
        norm_pending = None

        def flush_pv(now_slot):
            nonlocal pv_banks, norm_pending
            while pending and (now_slot is None
                               or pending[0][0] <= now_slot - PV_LAG):
                _, pst, ppair, ptile = pending.pop(0)
                if pst == 0 and ppair == 0:
                    if norm_pending is not None:
                        normalize_store(*norm_pending)
                        norm_pending = None
                    pv_banks = alloc_pv_banks()
                pv8(pst, ppair, ptile, pv_banks)
                if pst == ST - 1 and ppair == NPAIR - 1:
                    norm_pending = (pending_tch[0], pv_banks)
                    pending_tch[0] += 1

        pending_tch = [0]
        for tch in range(NTCH):
            for st in range(ST):
                slot = tch * ST + st
                s_ps_list = []
                for pair in range(NPAIR):
                    s_ps = spsum.tile([P, 2 * TCH], F32, tag="s", name="s_ps")
                    scores(tch, st, pair, s_ps)
                    s_ps_list.append(s_ps)
                if slot == 6:
                    strip_rot[0] = strip_rot[0][:4]
                for p_ in sched.pop(slot, ()):
                    p_()
                flush_pv(slot)
                for pair in range(NPAIR):
                    p_t = ppool.tile([P, 2 * TCH], BF16, tag="p", name="p_t")
                    nc.scalar.activation(out=p_t, in_=s_ps_list[pair],
                                         func=AF.Exp, scale=SCALE)
                    pending.append((slot, st, pair, p_t))
        flush_pv(None)
        normalize_store(*norm_pending)
        assert not sched, f"unconsumed splice slots: {sorted(sched)}"


def build_program():
    nc = bacc.Bacc("TRN2", target_bir_lowering=False, debug=False,
                   num_devices=N_CORES)

    x = nc.dram_tensor("x", [T, C], F32, kind="ExternalInput").ap()
    xe = nc.dram_tensor("xe", [S, C], F32, kind="ExternalInput").ap()
    wq = nc.dram_tensor("wq", [C, DCORE], F32, kind="ExternalInput").ap()
    wk = nc.dram_tensor("wk", [C, DCORE], F32, kind="ExternalInput").ap()
    wv = nc.dram_tensor("wv", [C, DCORE], F32, kind="ExternalInput").ap()
    out = nc.dram_tensor("out", [T, DCORE], F32, kind="ExternalOutput").ap()

    with tile.TileContext(nc) as tc:
        _build_body(nc, tc, x, xe, wq, wk, wv, out)
    nc.compile()
    return nc


_NC_CACHE = None


def _get_program():
    global _NC_CACHE
    if _NC_CACHE is None:
        _NC_CACHE = build_program()
    return _NC_CACHE


def kernel(x_enc, x, Wk, Wq, Wv):
    x_enc = np.asarray(x_enc, dtype=np.float32)
    x = np.asarray(x, dtype=np.float32)
    Wk = np.asarray(Wk, dtype=np.float32)
    Wq = np.asarray(Wq, dtype=np.float32)
    Wv = np.asarray(Wv, dtype=np.float32)

    nc = _get_program()
    in_maps = []
    for core in range(N_CORES):
        b, hg = divmod(core, N_CORES // B)
        csl = slice(hg * DCORE, (hg + 1) * DCORE)
        in_maps.append({
            "x": np.ascontiguousarray(x[b]),
            "xe": np.ascontiguousarray(x_enc[b]),
            "wq": np.ascontiguousarray(Wq[:, csl]),
            "wk": np.ascontiguousarray(Wk[:, csl]),
            "wv": np.ascontiguousarray(Wv[:, csl]),
        })
    res = run_bass_kernel_spmd(nc, in_maps, list(range(N_CORES)))

    full = np.empty((B, T, H, D), dtype=np.float32)
    for core in range(N_CORES):
        b, hg = divmod(core, N_CORES // B)
        o = res.results[core]["out"].reshape(T, HG, D)
        full[b, :, hg * HG:(hg + 1) * HG, :] = o
    return full
